# revision 14
# baseline (speedup 1.0000x reference)
"""DGRec kernel for 8 NeuronCores (Trainium2, Bass/Tile).

Strategy: the LSTM/GAT front-end is tiny (B=100, L=20, E=100) and is
replicated on every core; the dominant cost — logits = sr @ item_emb^T
producing a [100, 20, 30000] output (bf16 on the wire, 15MB/core) — is
sharded over the vocab dim (3750 vocab rows per core). No collectives.

The front-end is latency-bound (20 serial LSTM steps of small ops), so
the step is engineered for minimal dependency hops:
  - weights pre-scaled host-side so ONE Tanh over all 4 gate blocks
    yields [Ti|Tf|To|Tg] (sigmoid expressed via tanh keeps a single
    ACT table set: exp_and_others covers tanh/exp/relu/copy)
  - hidden state carried DOUBLED (h' = 2h); the 0.5 factors are folded
    into W_hh / W2a / W1b host-side
  - scalar_tensor_tensor fuses (1+T)*x into one DVE op
  - per-step chain: 4 whh MMs -> tanh(gates) -> STT -> add -> tanh ->
    STT = 6 dependency hops; c-state lives in the NEXT step's gate
    tile so [Tg | c] is one contiguous operand
Output tiles (psum -> bf16 SBUF -> DMA) stream behind the LSTM; DMAs
go out on the sync (SP) and gpsimd (SWDGE) queues to keep ACT clean.

Output is written l-major as out2d[l*100+b, v] and rearranged on host.
"""

import sys

import numpy as np

_TRN_REPO = "/opt/trn_rl_repo"
if _TRN_REPO not in sys.path:
    sys.path.insert(0, _TRN_REPO)

import concourse.bass as bass
import concourse.mybir as mybir
import concourse.tile as tile
from concourse import bacc, bass_utils

E = 100
B = 100
L = 20
V = 30000
NC = 8
VS = V // NC          # 3750 vocab rows per core
BL = B * L            # 2000
NU = 100000           # num users
S2 = 5
N2 = B * S2           # 500 layer2 support rows

F32 = mybir.dt.float32
F32R = mybir.dt.float32r   # fp32 bits, fast PE path (1 cyc/row at N>=256)
BF16 = mybir.dt.bfloat16   # output dtype: halves the 30MB/core store traffic
I32 = mybir.dt.int32


def _r(ap):
    return ap.bitcast(F32R)
AF = mybir.ActivationFunctionType
OP = mybir.AluOpType

# vocab chunking for the final matmul (<=512 free dim per psum bank)
CH = [(i * 512, min(512, VS - i * 512)) for i in range((VS + 511) // 512)]
# output row tiles (l-major rows of [BL, VS])
MT = [(m * 128, min(128, BL - m * 128)) for m in range((BL + 127) // 128)]

PROFILE = False
LAST_RESULTS = None

_CACHE = {}


def _emit(nc, tc, aps, reps=1, variant="full", tick=None):
    from contextlib import ExitStack

    (idx_all, mask_b, item_emb, user_emb, emb_t, w_ih, w_hh, w1ta, w1tb,
     w2ta, w2tb, wg0, ident, ones_d, chain, srt_in, out2d) = aps

    ctx = ExitStack()
    with ctx:
        consts = ctx.enter_context(tc.tile_pool(name="consts", bufs=1))
        state = ctx.enter_context(tc.tile_pool(name="state", bufs=1))
        gpool = ctx.enter_context(tc.tile_pool(name="gpool", bufs=3))
        work = ctx.enter_context(tc.tile_pool(name="work", bufs=2))
        gath = ctx.enter_context(tc.tile_pool(name="gath", bufs=6))
        outp = ctx.enter_context(tc.tile_pool(name="outp", bufs=2))
        ps_gates = ctx.enter_context(
            tc.tile_pool(name="ps_gates", bufs=2, space="PSUM"))
        ps_misc = ctx.enter_context(
            tc.tile_pool(name="ps_misc", bufs=2, space="PSUM"))
        ps_big = ctx.enter_context(
            tc.tile_pool(name="ps_big", bufs=2, space="PSUM"))

        if reps > 1:
            # timing harness: repeat the whole kernel body on-device;
            # the For_i back-edge is a full barrier between reps
            ctx.enter_context(tc.For_i(0, reps, 1))

        # ---------------- constants ----------------
        chain_sb = consts.tile([1, 512], F32)
        nc.sync.dma_start(chain_sb[:], chain)
        idx_sb = consts.tile([128, 28], I32)
        nc.sync.dma_start(idx_sb[:], idx_all)
        identity = consts.tile([128, 128], F32)
        nc.sync.dma_start(identity[:], ident)
        wih_sb = consts.tile([E + 1, 4 * E], F32)
        nc.sync.dma_start(wih_sb[:], w_ih)
        whh_sb = consts.tile([E, 4 * E], F32)
        nc.sync.dma_start(whh_sb[:], w_hh)
        w1ta_sb = consts.tile([E, E], F32)
        nc.sync.dma_start(w1ta_sb[:], w1ta)
        w1tb_sb = consts.tile([E, E], F32)
        nc.sync.dma_start(w1tb_sb[:], w1tb)
        w2ta_sb = consts.tile([E, E], F32)
        nc.sync.dma_start(w2ta_sb[:], w2ta)
        w2tb_sb = consts.tile([E, E], F32)
        nc.sync.dma_start(w2tb_sb[:], w2tb)
        wg0_sb = consts.tile([E, E], F32)
        nc.sync.dma_start(wg0_sb[:], wg0)
        # maskB / embT are large and not needed until the first sr/out-tile;
        # DMAs emitted after the gathers below so gathers don't queue behind
        maskB = consts.tile([B, BL], F32)
        embT = consts.tile([E, VS], F32R)

        # ---------------- persistent state ----------------
        nsteps = L
        if variant.startswith("nofinal") and variant != "nofinal":
            nsteps = int(variant[len("nofinal"):])

        xT = state.tile([E + 1, BL], F32)      # x transposed, col = t*B+b
        x0T = state.tile([E + 1, N2], F32)     # first item of support sess.
        lt2T = state.tile([E, N2], F32)        # user_emb gathers, transposed
        hT = state.tile([E, BL], F32)          # DOUBLED hidden h'=2h
        srT = state.tile([E, BL], F32R)        # masked sr (f32r), col = l*B+b
        ls2x = state.tile([E, 6 * B], F32)     # GAT neighbors, col = b*6+j
        pall = state.tile([E, 6 * B], F32)
        p2 = state.tile([E, 6 * B], F32)
        ctxT = state.tile([E, B], F32)
        h0T = state.tile([E, B], F32)
        h0r = state.tile([E, B], F32)          # real-scale h0 = 0.5*h'
        ones_col = state.tile([128, 1], F32)
        ones_row = state.tile([1, 128], F32)
        # per-step gate tiles: [0:400] = tanh(gates) [Ti|Tf|To|Tg] written
        # by ACT, [400:500] = c_{t-1} written by the previous step's DVE
        TTs = [gpool.tile([E, 500], F32, tag="tt", name=f"tt{t}")
               for t in range(nsteps)]

        nc.sync.dma_start(xT[E:E + 1, :], ones_d[:, 0:BL])
        nc.sync.dma_start(x0T[E:E + 1, :], ones_d[:, 0:N2])
        nc.gpsimd.memset(TTs[0][:, 400:500], 0.0)
        nc.gpsimd.memset(ones_col[:], 1.0)
        nc.gpsimd.memset(ones_row[:], 1.0)

        # alternate psum->sbuf copies between DVE and ACT
        tog = [0]

        def copy_out(dst, src):
            if tog[0] % 2 == 0:
                nc.vector.tensor_copy(dst, src)
            else:
                nc.scalar.activation(dst, src, AF.Copy)
            tog[0] += 1

        def gather_transpose(dst, src_table, col, n):
            """gather n rows of src_table by idx_all[:, col] -> dst [E, n]"""
            if variant == "hostgather":
                return
            g = gath.tile([128, E], F32, tag="g", name="g")
            nc.gpsimd.indirect_dma_start(
                out=g[0:n, :],
                out_offset=None,
                in_=src_table,
                in_offset=bass.IndirectOffsetOnAxis(
                    ap=idx_sb[0:n, col:col + 1], axis=0),
            )
            pt = ps_misc.tile([128, 512], F32, tag="m", name="pt")
            nc.tensor.transpose(
                out=pt[0:E, 0:n], in_=g[0:n, 0:E], identity=identity[0:n, 0:n])
            copy_out(dst, pt[0:E, 0:n])

        # session indices are packed l-major in 16 cols of 125 rows
        def gather_session(j):
            gather_transpose(xT[0:E, 125 * j:125 * (j + 1)], item_emb, j, 125)

        # all gathers issued upfront; SWDGE emits them ~back-to-back so
        # chunk j lands well ahead of the step that consumes it
        for j in range(16):
            gather_session(j)
        for k in range(4):
            gather_transpose(x0T[0:E, 125 * k:125 * (k + 1)], item_emb,
                             16 + k, 125)
        for k in range(4):
            gather_transpose(lt2T[0:E, 125 * k:125 * (k + 1)], user_emb,
                             20 + k, 125)
        nc.sync.dma_start(maskB[:], mask_b)
        nc.sync.dma_start(embT[:], emb_t)

        if variant == "hostgather":
            nc.sync.dma_start(xT[0:E, :], mask_b[0:E, :])
            nc.sync.dma_start(x0T[0:E, :], mask_b[0:E, 0:N2])
            nc.sync.dma_start(lt2T[:, :], mask_b[0:E, 0:N2])

        if variant.startswith("chain"):
            # micro: K dependent elementwise ops; "chainv"=DVE only,
            # "chainx"=alternating ACT/DVE (cross-engine hops)
            cw = work.tile([E, B], F32, tag="cw", name="cw")
            nc.gpsimd.memset(cw[:], 1.0)
            for i in range(256):
                if variant == "chainx" and i % 2 == 1:
                    nc.scalar.activation(cw[:], cw[:], AF.Copy)
                else:
                    nc.vector.tensor_scalar_add(cw[:], cw[:], 1.0)
            return

        if variant.startswith("mmchain"):
            # micro: K dependent matmul->copy round trips (PE <-> DVE)
            cw = work.tile([E, B], F32, tag="cw", name="cw")
            nc.gpsimd.memset(cw[:], 1.0)
            for i in range(128):
                mp = ps_misc.tile([128, 512], F32, tag="m", name="mcp")
                nc.tensor.matmul(out=mp[0:E, 0:B], lhsT=wg0_sb[:], rhs=cw[:],
                                 start=True, stop=True)
                nc.vector.tensor_copy(cw[:], mp[0:E, 0:B])
            return

        # ---------------- st2 / ls2 (support one-step LSTM + linear) -------
        # wih_sb gate blocks (host-permuted+prescaled): 0=i, 1=f, 2=o, 3=g.
        # f unused (c0=0).
        stp = {}
        for blk in (0, 2, 3):
            p = ps_misc.tile([128, 512], F32, tag="m", name="stp%d" % blk)
            nc.tensor.matmul(
                out=p[0:E, 0:N2],
                lhsT=wih_sb[:, 100 * blk:100 * (blk + 1)],
                rhs=x0T[:, :], start=True, stop=True)
            stp[blk] = p
        ti0 = work.tile([E, N2], F32, tag="ti0", name="ti0")
        to0 = work.tile([E, N2], F32, tag="to0", name="to0")
        tg0 = work.tile([E, N2], F32, tag="tg0", name="tg0")
        nc.scalar.activation(ti0[:], stp[0][0:E, 0:N2], AF.Tanh)
        nc.scalar.activation(to0[:], stp[2][0:E, 0:N2], AF.Tanh)
        nc.scalar.activation(tg0[:], stp[3][0:E, 0:N2], AF.Tanh)
        p0 = work.tile([E, N2], F32, tag="p0", name="p0")
        nc.vector.scalar_tensor_tensor(
            out=p0[:], in0=ti0[:], scalar=1.0, in1=tg0[:],
            op0=OP.add, op1=OP.mult)                      # (1+Ti)Tg = 2c0
        th0 = work.tile([E, N2], F32, tag="th0", name="th0")
        nc.scalar.activation(th0[:], p0[:], AF.Tanh, scale=0.5)  # tanh(c0)
        q0 = work.tile([E, N2], F32, tag="q0", name="q0")
        nc.vector.scalar_tensor_tensor(
            out=q0[:], in0=to0[:], scalar=1.0, in1=th0[:],
            op0=OP.add, op1=OP.mult)                      # (1+To)th0 = 2*st2
        lp = ps_misc.tile([128, 512], F32, tag="m", name="lp")
        nc.tensor.matmul(out=lp[0:E, 0:N2], lhsT=w1ta_sb[:],
                         rhs=lt2T[:], start=True, stop=False)
        nc.tensor.matmul(out=lp[0:E, 0:N2], lhsT=w1tb_sb[:],
                         rhs=q0[:], start=False, stop=True)  # w1tb
        # relu into strided columns of ls2x (col b*6+j, j<5)
        ls2x_v = ls2x.rearrange("p (b j) -> p b j", j=6)
        nc.scalar.activation(
            ls2x_v[:, :, 0:5],
            lp[0:E, 0:N2].rearrange("p (b j) -> p b j", j=5),
            AF.Relu)

        # ---------------- LSTM step ----------------
        def lstm_step(t):
            gp = ps_gates.tile([E, 4 * E], F32, tag="gates", name="gp")
            x_rhs = xT[:, B * t:B * (t + 1)]
            for blk in (3, 0, 1, 2):       # g first
                sl = slice(100 * blk, 100 * (blk + 1))
                nc.tensor.matmul(out=gp[:, sl], lhsT=wih_sb[:, sl], rhs=x_rhs,
                                 start=True, stop=(t == 0))
                if t > 0:
                    nc.tensor.matmul(out=gp[:, sl], lhsT=whh_sb[:, sl],
                                     rhs=hT[:, B * (t - 1):B * t],
                                     start=False, stop=True)
            TT = TTs[t]
            # one tanh over all gates (weights prescaled: ifo/2, g*1)
            nc.scalar.activation(TT[:, 0:400], gp[:, 0:400], AF.Tanh)
            prod = work.tile([E, 2 * B], F32, tag="prod", name="prod")
            nc.vector.scalar_tensor_tensor(
                out=prod[:], in0=TT[:, 0:200], scalar=1.0, in1=TT[:, 300:500],
                op0=OP.add, op1=OP.mult)     # [(1+Ti)Tg | (1+Tf)c]
            c2 = work.tile([E, B], F32, tag="c2", name="c2")
            nc.vector.tensor_tensor(out=c2[:], in0=prod[:, 0:B],
                                    in1=prod[:, B:2 * B], op=OP.add)  # 2c_t
            if t + 1 < nsteps:
                nc.vector.tensor_scalar_mul(TTs[t + 1][:, 400:500], c2[:],
                                            0.5)          # c_t (off-path)
            th = work.tile([E, B], F32, tag="th", name="th")
            nc.scalar.activation(th[:], c2[:], AF.Tanh, scale=0.5)
            nc.vector.scalar_tensor_tensor(
                out=hT[:, B * t:B * (t + 1)], in0=TT[:, 200:300], scalar=1.0,
                in1=th[:], op0=OP.add, op1=OP.mult)       # h' = (1+To)th

        # ---------------- GAT (after step 0) ----------------
        def gat():
            # h0r = real-scale h0 = 0.5 * h'
            nc.vector.tensor_scalar_mul(h0r[:], hT[:, 0:B], 0.5)
            nc.vector.tensor_copy(
                ls2x_v[:, :, 5:6],
                h0r.rearrange("p (b j) -> p b j", j=1))
            nc.vector.tensor_tensor(
                out=pall.rearrange("p (b j) -> p b j", j=6),
                in0=ls2x_v[:, :, :],
                in1=h0r.rearrange("p (b j) -> p b j", j=1).to_broadcast(
                    [E, B, 6]),
                op=OP.mult)
            # column sums via ones-matmul -> scores [1, 600]
            sp1 = ps_misc.tile([128, 512], F32, tag="m", name="sp1")
            nc.tensor.matmul(out=sp1[0:1, 0:512], lhsT=ones_col[0:E, 0:1],
                             rhs=pall[:, 0:512], start=True, stop=True)
            sp2 = ps_misc.tile([128, 512], F32, tag="m", name="sp2")
            nc.tensor.matmul(out=sp2[0:1, 0:88], lhsT=ones_col[0:E, 0:1],
                             rhs=pall[:, 512:600], start=True, stop=True)
            # softmax over j (6) per b on one partition; scores are O(0.1)
            # so max-subtraction is skipped; exp reads score psum directly
            erow = work.tile([1, 6 * B], F32, tag="erow", name="erow")
            nc.scalar.activation(erow[:, 0:512], sp1[0:1, 0:512], AF.Exp)
            nc.scalar.activation(erow[:, 512:600], sp2[0:1, 0:88], AF.Exp)
            esum = work.tile([1, B], F32, tag="esum", name="esum")
            nc.vector.tensor_reduce(
                out=esum[:], in_=erow.rearrange("p (b j) -> p b j", j=6),
                op=OP.add, axis=mybir.AxisListType.X)
            erec = work.tile([1, B], F32, tag="erec", name="erec")
            nc.vector.reciprocal(erec[:], esum[:])
            arow = work.tile([1, 6 * B], F32, tag="arow", name="arow")
            nc.vector.tensor_tensor(
                out=arow.rearrange("p (b j) -> p b j", j=6),
                in0=erow.rearrange("p (b j) -> p b j", j=6),
                in1=erec.rearrange("p (b j) -> p b j", j=1).to_broadcast(
                    [1, B, 6]),
                op=OP.mult)
            # broadcast alpha to all partitions via K=1 matmul
            ap1 = ps_misc.tile([128, 512], F32, tag="m", name="ap1")
            nc.tensor.matmul(out=ap1[0:E, 0:512], lhsT=ones_row[0:1, 0:E],
                             rhs=arow[0:1, 0:512], start=True, stop=True)
            ap2 = ps_misc.tile([128, 512], F32, tag="m", name="ap2")
            nc.tensor.matmul(out=ap2[0:E, 0:88], lhsT=ones_row[0:1, 0:E],
                             rhs=arow[0:1, 512:600], start=True, stop=True)
            # ctx = sum_j alpha * neighbors ; h0 = relu(Wg0.T @ ctxT)
            nc.vector.tensor_tensor(out=p2[:, 0:512], in0=ap1[0:E, 0:512],
                                    in1=ls2x[:, 0:512], op=OP.mult)
            nc.vector.tensor_tensor(out=p2[:, 512:600], in0=ap2[0:E, 0:88],
                                    in1=ls2x[:, 512:600], op=OP.mult)
            nc.vector.tensor_reduce(
                out=ctxT[:], in_=p2.rearrange("p (b j) -> p b j", j=6),
                op=OP.add, axis=mybir.AxisListType.X)
            hp = ps_misc.tile([128, 512], F32, tag="m", name="hp")
            nc.tensor.matmul(out=hp[0:E, 0:B], lhsT=wg0_sb[:], rhs=ctxT[:],
                             start=True, stop=True)
            nc.scalar.activation(h0T[:], hp[0:E, 0:B], AF.Relu)

        # ---------------- sr + output tiles ----------------
        def sr_step(t):
            sp = ps_misc.tile([128, 512], F32, tag="m", name="srp")
            nc.tensor.matmul(out=sp[0:E, 0:B], lhsT=w2ta_sb[:],
                             rhs=hT[:, B * t:B * (t + 1)], start=True,
                             stop=False)                  # w2ta prescaled .5
            nc.tensor.matmul(out=sp[0:E, 0:B], lhsT=w2tb_sb[:], rhs=h0T[:],
                             start=False, stop=True)
            nc.vector.tensor_tensor(out=srT[:, B * t:B * (t + 1)],
                                    in0=sp[0:E, 0:B],
                                    in1=maskB[:, B * t:B * (t + 1)],
                                    op=OP.mult)

        def out_tile(m):
            base, rows = MT[m]
            ot = outp.tile([128, VS], BF16, tag="out", name="ot")
            # pairs of 512-col matmuls into a 2-bank psum tile, one copy per
            # pair (halves the per-copy fixed overhead)
            for pi in range(0, len(CH), 2):
                pair = CH[pi:pi + 2]
                w = sum(cn for _, cn in pair)
                bp = ps_big.tile([128, 1024], F32, tag="big", name="bp")
                for (c0, cn) in pair:
                    nc.tensor.matmul(out=bp[0:rows, c0 - pair[0][0]:
                                            c0 - pair[0][0] + cn],
                                     lhsT=srT[:, base:base + rows],
                                     rhs=embT[:, c0:c0 + cn], start=True,
                                     stop=True)
                copy_out(ot[0:rows, pair[0][0]:pair[0][0] + w],
                         bp[0:rows, 0:w])
            # out DMAs on SP (HWDGE) and gpsimd (SWDGE) queues; ACT's
            # HWDGE ring is left for the LSTM loop's activations
            eng = nc.sync if m % 2 == 0 else nc.gpsimd
            eng.dma_start(out2d[base:base + rows, :], ot[0:rows, :])

        if variant == "outonly":
            nc.sync.dma_start(srT[:], srt_in)
            for m in range(len(MT)):
                out_tile(m)
            return

        # ---------------- main loop ----------------
        emitted = 0
        for t in range(nsteps):
            lstm_step(t)
            if t == 0:
                gat()
            sr_step(t)
            if variant.startswith("nofinal"):
                continue
            # first tiles emit eagerly (startup); later tiles lag one step
            lag = 0 if emitted < 3 else 1
            while emitted < len(MT) and (
                    MT[emitted][0] + MT[emitted][1] <= B * (t + 1 - lag)):
                out_tile(emitted)
                emitted += 1
        while emitted < len(MT) and not variant.startswith("nofinal"):
            out_tile(emitted)
            emitted += 1
        if variant.startswith("nofinal"):
            # still produce one output tile so the NEFF has output traffic
            out_tile(0)

        if tick is not None:
            nc.sync.dma_start(tick, chain_sb[0:1, 0:1])


# tensors made Internal (device scratch) in timing builds so repeated runs
# don't ship ~650MB through the axon tunnel per call
_BIG = {"mask_b", "item_emb", "user_emb", "emb_t", "srt_in", "out2d"}


def _build(reps=1, variant="full", timing=False):
    key = ("nc", reps, variant, timing)
    if key in _CACHE:
        return _CACHE[key]
    nc = bacc.Bacc("TRN2", target_bir_lowering=False, debug=False,
                   enable_asserts=False, num_devices=NC)

    def kind(name, k):
        if timing and name in _BIG:
            return "Internal"
        return k

    aps = (
        nc.dram_tensor("idx_all", [128, 28], I32, kind="ExternalInput").ap(),
        nc.dram_tensor("mask_b", [B, BL], F32,
                       kind=kind("mask_b", "ExternalInput")).ap(),
        nc.dram_tensor("item_emb", [V, E], F32,
                       kind=kind("item_emb", "ExternalInput")).ap(),
        nc.dram_tensor("user_emb", [NU, E], F32,
                       kind=kind("user_emb", "ExternalInput")).ap(),
        nc.dram_tensor("emb_t", [E, VS], F32R,
                       kind=kind("emb_t", "ExternalInput")).ap(),
        nc.dram_tensor("w_ih", [E + 1, 4 * E], F32,
                       kind="ExternalInput").ap(),
        nc.dram_tensor("w_hh", [E, 4 * E], F32, kind="ExternalInput").ap(),
        nc.dram_tensor("w1ta", [E, E], F32, kind="ExternalInput").ap(),
        nc.dram_tensor("w1tb", [E, E], F32, kind="ExternalInput").ap(),
        nc.dram_tensor("w2ta", [E, E], F32, kind="ExternalInput").ap(),
        nc.dram_tensor("w2tb", [E, E], F32, kind="ExternalInput").ap(),
        nc.dram_tensor("wg0", [E, E], F32, kind="ExternalInput").ap(),
        nc.dram_tensor("ident", [128, 128], F32, kind="ExternalInput").ap(),
        nc.dram_tensor("ones_d", [1, BL], F32, kind="ExternalInput").ap(),
        nc.dram_tensor("chain", [1, 512], F32, kind="ExternalInput").ap(),
        nc.dram_tensor("srt_in", [E, BL], F32R,
                       kind=kind("srt_in", "ExternalInput")).ap(),
        nc.dram_tensor("out2d", [BL, VS], BF16,
                       kind=kind("out2d", "ExternalOutput")).ap(),
    )
    tick = None
    if timing:
        tick = nc.dram_tensor("tick", [1, 1], F32, kind="ExternalOutput").ap()
    with tile.TileContext(nc) as tc:
        _emit(nc, tc, aps, reps=reps, variant=variant, tick=tick)
    nc.compile()
    _CACHE[key] = nc
    return nc


def make_in_maps(**inputs):
    ins = {k: np.asarray(v) for k, v in inputs.items()}
    item_emb = np.ascontiguousarray(ins["item_emb"], np.float32)
    user_emb = np.ascontiguousarray(ins["user_emb"], np.float32)
    sess = ins["input_session"].astype(np.int32)
    ss2 = ins["support_sessions_layer2"].astype(np.int32)
    sn2 = ins["support_nodes_layer2"].astype(np.int32)
    mask = ins["mask_y"].astype(np.float32)
    W_ih = ins["W_ih"].astype(np.float32)
    W_hh = ins["W_hh"].astype(np.float32)
    b_ih = ins["b_ih"].astype(np.float32)
    b_hh = ins["b_hh"].astype(np.float32)
    W1 = ins["W1"].astype(np.float32)
    W2 = ins["W2"].astype(np.float32)
    Wg0 = ins["Wg0"].astype(np.float32)

    # permute gate blocks from pytorch order [i, f, g, o] to [i, f, o, g]
    perm = np.concatenate([np.arange(0, 100), np.arange(100, 200),
                           np.arange(300, 400), np.arange(200, 300)])
    # prescale so one tanh yields [Ti|Tf|To|Tg] = tanh([i/2,f/2,o/2,g]);
    # W_hh additionally *0.5 since the carried hidden state is h'=2h
    gs = np.concatenate([np.full(300, 0.5, np.float32),
                         np.ones(100, np.float32)])
    wih_aug = np.concatenate(
        [W_ih[perm].T, (b_ih + b_hh)[perm][None, :]], axis=0) * gs[None, :]
    whh_p = np.ascontiguousarray(W_hh[perm].T * (gs * 0.5)[None, :])
    w1t = W1.T                                                 # [200, 100]
    idx_all = np.zeros((128, 28), np.int32)
    idx_all[0:125, 0:16] = sess.T.reshape(16, 125).T   # l-major session idx
    idx_all[0:125, 16:20] = ss2[:, 0].reshape(4, 125).T
    idx_all[0:125, 20:24] = sn2.reshape(4, 125).T
    mask_b = np.ascontiguousarray(
        np.broadcast_to(mask.T.reshape(1, BL), (B, BL)))
    base = {
        "idx_all": idx_all,
        "mask_b": mask_b,
        "item_emb": item_emb,
        "user_emb": user_emb,
        "w_ih": np.ascontiguousarray(wih_aug),
        "w_hh": whh_p,
        "w1ta": np.ascontiguousarray(w1t[0:100]),
        "w1tb": np.ascontiguousarray(w1t[100:200] * 0.5),  # ls2 uses 2*st2
        "w2ta": np.ascontiguousarray(W2[:, 0:100].T * 0.5),  # sr uses h'=2h
        "w2tb": np.ascontiguousarray(W2[:, 100:200].T),
        "wg0": np.ascontiguousarray(Wg0),
        "ident": np.eye(128, dtype=np.float32),
        "ones_d": np.ones((1, BL), np.float32),
        "chain": np.zeros((1, 512), np.float32),
        "srt_in": np.zeros((E, BL), np.float32),
    }
    in_maps = []
    for c in range(NC):
        m = dict(base)
        m["emb_t"] = np.ascontiguousarray(item_emb[c * VS:(c + 1) * VS].T)
        in_maps.append(m)
    return in_maps


def make_timing_in_maps(**inputs):
    """Small-tensor-only in_maps for timing builds (_BIG are Internal)."""
    m = {k: v for k, v in make_in_maps(**inputs)[0].items() if k not in _BIG}
    return [dict(m) for _ in range(NC)]


def kernel(**inputs):
    global LAST_RESULTS
    in_maps = make_in_maps(**inputs)
    nc = _build()
    res = bass_utils.run_bass_kernel_spmd(
        nc, in_maps, core_ids=list(range(NC)))
    LAST_RESULTS = res
    full = np.empty((B, L, V), np.float32)
    for c in range(NC):
        o = np.asarray(res.results[c]["out2d"]).astype(np.float32)
        full[:, :, c * VS:(c + 1) * VS] = o.reshape(L, B, VS).transpose(
            1, 0, 2)
    return full
